# revision 2
# baseline (speedup 1.0000x reference)
"""Cross-attention layer (B=2, L=2048, D=1024, 16 heads) on 8 TRN2 NeuronCores.

Single-phase fp8 design. Core c handles batch b=c//4 and q rows
512*(c%4)..512*(c%4+1). Each core redundantly projects K,V for its whole
batch in fp8 DoubleRow matmuls (hidden under the softmax exp, which
saturates the Activation engine), then runs scores / ctx / output
projection also as fp8 DoubleRow.

Numerics (validated vs f64 reference on host, max_rel ~1e-3 << 2e-2):
  - all matmul operands fp8e4m3; psum accumulation f32
  - bk dropped entirely (softmax is shift-invariant along kv)
  - bv folded in post-normalize (sum of attention weights == 1)
  - ctx scaled by 16 before fp8 quantization (avoids e4m3 denormals);
    Wo pre-divided by 16 on host; the 16 comes free because the V
    augmentation column holds 1/16 (denominator = den/16)
DoubleRow ISA constraints found by probing walrus: output tile_position
column must be 0 and M must be a multiple of 16 — hence the ones-
augmented V padded to M=80 (64 ctx dims + den row + 15 zero rows), one
psum bank per head, and a 64-partition ctxt layout (one head per slot)
so nothing ever needs a partition remap.
Scores trick: head-dim contraction (64) is split [32, 2] with d in
[0,32) on partitions 32*(h%4).. and d in [32,64) at DR free offset j=1;
the K/Q projections produce this layout directly via host-permuted
weight columns (row tile_position offsets are legal in DR mode).
"""

import numpy as np
import ml_dtypes

import concourse.mybir as mybir
import concourse.tile as tile
from concourse import bacc
from concourse.bass_utils import run_bass_kernel_spmd

dt = mybir.dt
AF = mybir.ActivationFunctionType
ALU = mybir.AluOpType
DR = mybir.MatmulPerfMode.DoubleRow
F8 = ml_dtypes.float8_e4m3

P = 128
B, LQ, LKV = 2, 2048, 2048
DQ, DKV, HID, NH = 1024, 1024, 1024, 16
HD = HID // NH
EPS = 1e-5
N_CORES = 8
RQ = LQ * B // N_CORES             # 512 q rows per core
NPAIR = NH // 2                    # 8 head pairs
KV_C = LKV // P                    # 16 kv chunks of 128
KV_N = LKV // 512                  # 4 kv slices of 512
MQ = RQ // P                       # 4 q tiles of 128
VA = 80                            # augmented V columns (64 + den + pad)


def build():
    nc = bacc.Bacc("TRN2", target_bir_lowering=False, debug=False,
                   num_devices=N_CORES)
    f32, f8 = dt.float32, dt.float8e4
    f32r = dt.float32r

    kvt_d = nc.dram_tensor("kvt", [P, KV_N * 8 * 512], f8, kind="ExternalInput")
    qt_d = nc.dram_tensor("qt", [P, 8 * RQ], f8, kind="ExternalInput")
    wq_d = nc.dram_tensor("wq", [P, 8 * 4 * 2 * P], f8, kind="ExternalInput")
    wk_d = nc.dram_tensor("wk", [P, 8 * 4 * 2 * P], f8, kind="ExternalInput")
    wv_d = nc.dram_tensor("wv", [P, 2 * 4 * 2 * 512], f8, kind="ExternalInput")
    wo_d = nc.dram_tensor("wo", [64, 8 * 2 * DQ], f8, kind="ExternalInput")
    bq_d = nc.dram_tensor("bq", [P, 8], f32, kind="ExternalInput")
    bv_d = nc.dram_tensor("bv", [64, NH], f32, kind="ExternalInput")
    xq_d = nc.dram_tensor("xq", [RQ, DQ], f32, kind="ExternalInput")
    gam_d = nc.dram_tensor("gamma", [1, DQ], f32r, kind="ExternalInput")
    bet_d = nc.dram_tensor("beta", [1, DQ], f32r, kind="ExternalInput")
    out_d = nc.dram_tensor("out", [RQ, DQ], f32, kind="ExternalOutput")

    kvt_r = kvt_d.ap().rearrange("p (n c k) -> n p c k", n=KV_N, c=8)
    qt_r = qt_d.ap().rearrange("p (c k) -> p c k", c=8)
    wq_r = wq_d.ap().rearrange("p (x t i m) -> x p t i m", x=8, t=4, i=2)
    wk_r = wk_d.ap().rearrange("p (x t i m) -> x p t i m", x=8, t=4, i=2)
    wv_r = wv_d.ap().rearrange("p (x t i m) -> x p t i m", x=2, t=4, i=2)
    wo_r = wo_d.ap().rearrange("p (t i m) -> p t i m", t=8, i=2)
    xq_r = xq_d.ap().rearrange("(m p) e -> m p e", p=P)
    out_r = out_d.ap().rearrange("(m p) e -> m p e", p=P)

    with tile.TileContext(nc) as tc:
        with (
            tc.tile_pool(name="const", bufs=1) as const,
            tc.tile_pool(name="epool", bufs=3) as epool,
            tc.tile_pool(name="nm", bufs=2) as nm,
            tc.tile_pool(name="sm", bufs=2) as sm,
            tc.tile_pool(name="opool", bufs=2) as opool,
            tc.tile_pool(name="ps_sc", bufs=2, space="PSUM") as ps_sc,
            tc.tile_pool(name="ctx_ps", bufs=2, space="PSUM") as ctx_ps,
            tc.tile_pool(name="proj_ps", bufs=2, space="PSUM") as proj_ps,
        ):
            # ---- resident SBUF tensors ----
            kvt_sb = const.tile([P, KV_N, 8, 512], f8)
            qt_sb = const.tile([P, 8, RQ], f8)
            wq_sb = const.tile([P, 8, 4, 2, P], f8)
            wk_sb = const.tile([P, 8, 4, 2, P], f8)
            wv_sb = const.tile([P, 2, 4, 2, 512], f8)
            wo_sb = const.tile([64, 8, 2, DQ], f8)
            k_sb = const.tile([P, 4, 2, LKV], f8)       # [32a+d', hg, j, kv]
            q_sb = const.tile([P, 4, 2, RQ], f8)        # [32a+d', hg, j, q]
            v_sb = const.tile([P, 8, 2, NH, VA], f8)    # [kv%128, pr, hf, h, .]
            ctxt_sb = const.tile([64, NH, RQ], f8)      # [d, h, q] (scaled 16x)
            xq_sb = const.tile([P, MQ, DQ], f32)
            bq_sb = const.tile([P, 8], f32)
            bv_sb = const.tile([64, NH], f32)           # 16*bv, [d, h]
            gb_bc = const.tile([P, 2, DQ], f32)
            eps_t = const.tile([P, 1], f32)
            nc.vector.memset(eps_t[:], EPS)

            # ---- DMAs in priority order (serialize on the DMA engines) ----
            nc.sync.dma_start(kvt_sb[:, 0, 0:2], kvt_r[0][:, 0:2])
            nc.sync.dma_start(wk_sb[:, 0], wk_r[0])
            nc.sync.dma_start(qt_sb[:, 0:2], qt_r[:, 0:2])
            nc.sync.dma_start(wq_sb[:, 0], wq_r[0])
            nc.sync.dma_start(bq_sb[:], bq_d.ap())
            nc.sync.dma_start(kvt_sb[:, 0, 2:8], kvt_r[0][:, 2:8])
            nc.sync.dma_start(qt_sb[:, 2:8], qt_r[:, 2:8])
            nc.sync.dma_start(wk_sb[:, 1], wk_r[1])
            nc.sync.dma_start(wq_sb[:, 1], wq_r[1])
            nc.sync.dma_start(wv_sb[:, 0], wv_r[0])
            for n in range(1, KV_N):
                nc.sync.dma_start(kvt_sb[:, n], kvt_r[n])
            for x in range(2, 8):
                nc.sync.dma_start(wq_sb[:, x], wq_r[x])
                nc.sync.dma_start(wk_sb[:, x], wk_r[x])
            nc.sync.dma_start(wv_sb[:, 1], wv_r[1])
            nc.sync.dma_start(bv_sb[:], bv_d.ap())
            nc.sync.dma_start(wo_sb[:], wo_r)
            for m in range(MQ):
                nc.sync.dma_start(xq_sb[:, m], xq_r[m])
            for i, rd in enumerate((gam_d, bet_d)):
                row = sm.tile([1, DQ], f32r, tag="gbrow", name=f"gbrow{i}")
                nc.sync.dma_start(row[:], rd.ap())
                nc.gpsimd.partition_broadcast(gb_bc[:, i, :],
                                              row[:].bitcast(f32))

            # ---- projection emitters ----
            def q_proj(hg, j, pool):
                idx = 2 * hg + j
                ps = pool.tile([P, 512], f32, tag="sc", name=f"q{hg}{j}")
                for t in range(4):
                    nc.tensor.matmul(ps, wq_sb[:, idx, t],
                                     qt_sb[:, 2 * t:2 * t + 2],
                                     start=(t == 0), stop=(t == 3), perf_mode=DR)
                nc.vector.tensor_scalar(q_sb[:, hg, j], ps,
                                        bq_sb[:, idx:idx + 1], None, op0=ALU.add)

            def k_proj(hg, j, n, pool):
                idx = 2 * hg + j
                ps = pool.tile([P, 512], f32, tag="sc", name=f"k{hg}{j}{n}")
                for t in range(4):
                    nc.tensor.matmul(ps, wk_sb[:, idx, t],
                                     kvt_sb[:, n, 2 * t:2 * t + 2],
                                     start=(t == 0), stop=(t == 3), perf_mode=DR)
                nc.vector.tensor_copy(k_sb[:, hg, j, 512 * n:512 * (n + 1)], ps)

            def v_proj(ch, colh, pool):
                n, r = divmod(ch, 4)
                ps = pool.tile([P, 512], f32, tag="sc", name=f"v{ch}{colh}")
                for t in range(4):
                    nc.tensor.matmul(
                        ps, kvt_sb[:, n, 2 * t:2 * t + 2, P * r:P * (r + 1)],
                        wv_sb[:, colh, t], start=(t == 0), stop=(t == 3),
                        perf_mode=DR)
                nc.vector.tensor_copy(
                    v_sb[:, ch // 2, ch % 2, 8 * colh:8 * (colh + 1), 0:64], ps)

            # ---- prologue: minimum to start attention on hp0 ----
            k_proj(0, 0, 0, proj_ps)
            q_proj(0, 0, ps_sc)
            q_proj(0, 1, ps_sc)
            k_proj(0, 1, 0, proj_ps)
            # V augmentation columns: 64 = 1/16 (denominator), 65.. = 0
            # (on Pool so they don't block the prologue psum copies on DVE)
            for pr8 in range(8):
                for hf in range(2):
                    nc.gpsimd.memset(v_sb[:, pr8, hf, :, 64:VA], 0.0)
                    nc.gpsimd.memset(v_sb[:, pr8, hf, :, 64:65], 1.0 / 16.0)

            # remaining projection work, need-ordered, drained per score
            # batch with a deadline-verified quota schedule
            pending = []
            pending += [lambda ch=ch: v_proj(ch, 0, proj_ps) for ch in (0, 1)]
            pending += [lambda: k_proj(0, 0, 1, proj_ps),
                        lambda: k_proj(0, 1, 1, proj_ps)]
            pending += [lambda ch=ch: v_proj(ch, 0, proj_ps)
                        for ch in (2, 3, 4, 5)]
            pending += [lambda: k_proj(0, 0, 2, proj_ps),
                        lambda: k_proj(0, 1, 2, proj_ps)]
            pending += [lambda ch=ch: v_proj(ch, 0, proj_ps)
                        for ch in (6, 7, 8, 9)]
            pending += [lambda: k_proj(0, 0, 3, proj_ps),
                        lambda: k_proj(0, 1, 3, proj_ps)]
            pending += [lambda ch=ch: v_proj(ch, 0, proj_ps)
                        for ch in (10, 11, 12, 13, 14, 15)]
            for hg in range(1, 4):
                pending += [lambda hg=hg: q_proj(hg, 0, proj_ps),
                            lambda hg=hg: q_proj(hg, 1, proj_ps)]
                for n in range(KV_N):
                    pending += [lambda hg=hg, n=n: k_proj(hg, 0, n, proj_ps),
                                lambda hg=hg, n=n: k_proj(hg, 1, n, proj_ps)]
                if hg >= 2:
                    lo = 6 if hg == 3 else 0
                    pending += [lambda ch=ch: v_proj(ch, 1, proj_ps)
                                for ch in range(lo, lo + 6 if hg == 2 else KV_C)]
            pi = [0]
            quota = ([2, 2, 2, 2, 2, 2, 1, 1, 2, 1, 1, 1, 1, 1, 1, 1]
                     + [1] * 112)

            def drain_pending(k):
                for _ in range(k):
                    if pi[0] < len(pending):
                        pending[pi[0]]()
                        pi[0] += 1

            # ---- attention: sequential head pairs; ctx matmuls delayed one
            # score batch so they never separate an unfinished exp from the
            # next scores in the in-order PE stream ----
            batch = 0
            for hp in range(NPAIR):
                hg = hp // 2
                ctxs = [ctx_ps.tile([VA, RQ], f32, tag="ctx", name=f"ctx{s}")
                        for s in range(2)]
                ctx_q = []

                def emit_ctx(pr, e_t, hp=hp, ctxs=ctxs):
                    for hh in range(2):
                        h = 2 * hp + hh
                        nc.tensor.matmul(
                            ctxs[hh][:], v_sb[:, pr, :, h],
                            e_t[:, hh], start=(pr == 0), stop=(pr == 7),
                            perf_mode=DR)

                for pr in range(8):
                    e_t = epool.tile([P, 2, 2, RQ], f8, tag="e")
                    for cpar in range(2):
                        c = 2 * pr + cpar
                        ps_s = ps_sc.tile([P, 2, RQ], f32, tag="sc")
                        for hh in range(2):
                            a = (2 * hp + hh) % 4
                            nc.tensor.matmul(
                                ps_s[:, hh],
                                k_sb[32 * a:32 * (a + 1), hg, :,
                                     P * c:P * (c + 1)],
                                q_sb[32 * a:32 * (a + 1), hg],
                                start=True, stop=True, perf_mode=DR,
                                tile_position=(32 * a, 0))
                        nc.scalar.activation(e_t[:, :, cpar], ps_s[:],
                                             AF.Exp, scale=1.0 / np.sqrt(HD))
                        if ctx_q:
                            ctx_q.pop(0)()
                        drain_pending(quota[batch])
                        batch += 1
                        if cpar == 1:
                            ctx_q.append(
                                lambda pr=pr, e_t=e_t: emit_ctx(pr, e_t))
                for fn in ctx_q:
                    fn()

                # normalize: ctxt = ctx * (16/den) + 16*bv, quantized fp8.
                # All at partitions 0..63; stage-wise so the ctx psum banks
                # free quickly (copies+recips first) for the next head pair.
                cps, recs, rbs = [], [], []
                for hh in range(2):
                    rec = sm.tile([1, RQ], f32, tag="rec", name=f"rec{hh}",
                                  bufs=4)
                    nc.vector.reciprocal(rec[:], ctxs[hh][64:65, :])
                    recs.append(rec)
                for hh in range(2):
                    rb = nm.tile([64, RQ], f32, tag="rb", name=f"rb{hh}")
                    nc.gpsimd.partition_broadcast(rb[:], recs[hh][:])
                    rbs.append(rb)
                    cp = nm.tile([64, RQ], f32, tag="cp", name=f"cp{hh}")
                    nc.vector.tensor_copy(cp[:], ctxs[hh][0:64, :])
                    cps.append(cp)
                for hh in range(2):
                    h = 2 * hp + hh
                    tmp = nm.tile([64, RQ], f32, tag="tmp", name=f"tmp{hh}")
                    nc.vector.tensor_tensor(tmp[:], cps[hh][:], rbs[hh][:],
                                            op=ALU.mult)
                    nc.vector.tensor_scalar(ctxt_sb[:, h, :], tmp[:],
                                            bv_sb[:, h:h + 1], None,
                                            op0=ALU.add)

            # ---- output projection + residual + LayerNorm ----
            # four concurrent psum groups (borrowing the idle proj/ctx pools)
            # so all four q tiles project and LN-pipeline together
            xs, mus, m2s, vars_ = [], [], [], []
            opg = {}
            opg[0] = [ps_sc.tile([P, 2, 512], f32, tag="sc", name="ops0")]
            opg[1] = [ps_sc.tile([P, 2, 512], f32, tag="sc", name="ops1")]
            opg[2] = [proj_ps.tile([P, 512], f32, tag="sc", name=f"ops2{n}")
                      for n in range(2)]
            opg[3] = [ctx_ps.tile([P, RQ], f32, tag="ctx", name=f"ops3{n}")
                      for n in range(2)]
            def ops_ap(m, n2):
                g = opg[m]
                return g[0][:, n2] if len(g) == 1 else g[n2][:]
            for t in range(8):
                for m in range(MQ):
                    for n2 in range(2):
                        nc.tensor.matmul(
                            ops_ap(m, n2),
                            ctxt_sb[:, 2 * t:2 * t + 2, P * m:P * (m + 1)],
                            wo_sb[:, t, :, 512 * n2:512 * (n2 + 1)],
                            start=(t == 0), stop=(t == 7), perf_mode=DR)
            for m in range(MQ):
                x = opool.tile([P, DQ], f32, tag="x", name=f"x{m}", bufs=4)
                mu = sm.tile([P, 1], f32, tag="mu", name=f"mu{m}", bufs=4)
                if len(opg[m]) == 1:
                    src_ap = opg[m][0][:].rearrange("p a b -> p (a b)")
                    nc.vector.scalar_tensor_tensor(
                        x[:], src_ap, 1.0, xq_sb[:, m], op0=ALU.mult,
                        op1=ALU.add, accum_out=mu[:])
                else:
                    mu2 = sm.tile([P, 1], f32, tag="mu2", name=f"mu2{m}",
                                  bufs=2)
                    for n2 in range(2):
                        nc.vector.scalar_tensor_tensor(
                            x[:, 512 * n2:512 * (n2 + 1)], ops_ap(m, n2), 1.0,
                            xq_sb[:, m, 512 * n2:512 * (n2 + 1)],
                            op0=ALU.mult, op1=ALU.add,
                            accum_out=(mu if n2 == 0 else mu2)[:])
                    nc.vector.tensor_tensor(mu[:], mu[:], mu2[:], op=ALU.add)
                xs.append(x)
                mus.append(mu)
            for m in range(MQ):
                xx = opool.tile([P, DQ], f32, tag="xx", name=f"xx{m}", bufs=1)
                m2 = sm.tile([P, 1], f32, tag="m2", name=f"m2{m}", bufs=4)
                nc.scalar.activation(xx[:], xs[m][:], AF.Square,
                                     accum_out=m2[:])
                nc.vector.tensor_scalar(mus[m][:], mus[m][:], 1.0 / DQ, None,
                                        op0=ALU.mult)
                m2s.append(m2)
            for m in range(MQ):
                musq = sm.tile([P, 1], f32, tag="musq", name=f"musq{m}", bufs=4)
                nc.vector.tensor_tensor(musq[:], mus[m][:], mus[m][:],
                                        op=ALU.mult)
                var = sm.tile([P, 1], f32, tag="var", name=f"var{m}", bufs=4)
                nc.vector.tensor_scalar(var[:], m2s[m][:], 1.0 / DQ, None,
                                        op0=ALU.mult)
                nc.vector.tensor_tensor(var[:], var[:], musq[:],
                                        op=ALU.subtract)
                sd = sm.tile([P, 1], f32, tag="sd", name=f"sd{m}", bufs=4)
                nc.scalar.activation(sd[:], var[:], AF.Sqrt, bias=eps_t[:])
                vars_.append(sd)
            for m in range(MQ):
                rstd = sm.tile([P, 1], f32, tag="rstd", name=f"rstd{m}", bufs=4)
                nc.vector.reciprocal(rstd[:], vars_[m][:])
                y = opool.tile([P, DQ], f32, tag="y", name=f"y{m}", bufs=2)
                nc.vector.scalar_tensor_tensor(
                    y[:], xs[m][:], mus[m][:], gb_bc[:, 0], op0=ALU.subtract,
                    op1=ALU.mult)
                z2 = opool.tile([P, DQ], f32, tag="z2", name=f"z2{m}", bufs=2)
                nc.vector.scalar_tensor_tensor(
                    z2[:], y[:], rstd[:], gb_bc[:, 1], op0=ALU.mult,
                    op1=ALU.add)
                nc.sync.dma_start(out_r[m], z2[:])

    nc.compile()
    return nc


_CACHE = {}


def _get():
    if "nc" not in _CACHE:
        _CACHE["nc"] = build()
    return _CACHE["nc"]


def _f8(x):
    return np.ascontiguousarray(np.asarray(x, np.float32)).astype(F8)


def _dr_rows(w):
    """[1024, N] -> [128, 4(t), 2(i), N]: row 128*(2t+i)+p -> (p, t, i)."""
    return np.ascontiguousarray(
        w.reshape(4, 2, P, -1).transpose(2, 0, 1, 3))


# column permutation for the K/Q head-split layout:
# col (hg, j, a, d') = 64*(4*hg+a) + 32*j + d'
_PERM = (64 * (4 * np.arange(4)[:, None, None, None]
               + np.arange(4)[None, None, :, None])
         + 32 * np.arange(2)[None, :, None, None]
         + np.arange(32)[None, None, None, :]).reshape(4, 2, P)


def kernel(query, key_value, Wq, bq, Wk, bk, Wv, bv, Wo, bo, ln_gamma, ln_beta):
    query = np.asarray(query, np.float32)
    key_value = np.asarray(key_value, np.float32)
    Wq = np.asarray(Wq, np.float32)
    Wk = np.asarray(Wk, np.float32)
    Wv = np.asarray(Wv, np.float32)
    Wo = np.asarray(Wo, np.float32)
    bq = np.asarray(bq, np.float32)
    bv = np.asarray(bv, np.float32)
    bo = np.asarray(bo, np.float32)

    # weights in DoubleRow layouts (host-side, shared by all cores)
    wq_h = _dr_rows(Wq)[:, :, :, _PERM]      # [128, 4, 2, 4, 2, 128]
    wq_h = _f8(wq_h.transpose(0, 3, 4, 1, 2, 5).reshape(P, -1))
    wk_h = _dr_rows(Wk)[:, :, :, _PERM]
    wk_h = _f8(wk_h.transpose(0, 3, 4, 1, 2, 5).reshape(P, -1))
    wv_h = _f8(_dr_rows(Wv).reshape(P, 4, 2, 2, 512)
               .transpose(0, 3, 1, 2, 4).reshape(P, -1))
    # wo rows: [64(d), 8(t), 2(i), 1024] with hid = 64*(2t+i)+d, 1/16 scaled
    wo_h = _f8((Wo / 16.0).reshape(8, 2, 64, DQ)
               .transpose(2, 0, 1, 3).reshape(64, -1))
    # bq in psum layout: (p, idx=(hg,j)) -> bq[256*hg + 64*(p//32) + 32*j + p%32]
    p = np.arange(P)
    hgj = np.arange(8)
    bq_h = np.ascontiguousarray(
        bq[256 * (hgj[None, :] // 2) + 64 * (p[:, None] // 32)
           + 32 * (hgj[None, :] % 2) + (p[:, None] % 32)])
    bv_h = np.ascontiguousarray(16.0 * bv.reshape(NH, 64).T)
    gam_h = np.ascontiguousarray(ln_gamma, np.float32).reshape(1, DQ)
    bet_h = np.ascontiguousarray(ln_beta, np.float32).reshape(1, DQ)

    in_maps = []
    for c in range(N_CORES):
        b, rq = divmod(c, N_CORES // B)
        rows = slice(RQ * rq, RQ * (rq + 1))
        kvt = _f8(key_value[b].T)            # [1024, 2048]
        # [p, n, c8, k'] with dkv = 128*c8 + p, kv = 512*n + k'
        kvt_h = np.ascontiguousarray(
            kvt.reshape(8, P, KV_N, 512).transpose(1, 2, 0, 3).reshape(P, -1))
        qt = _f8(query[b, rows].T)           # [1024, 512]
        qt_h = np.ascontiguousarray(
            qt.reshape(8, P, RQ).transpose(1, 0, 2).reshape(P, -1))
        in_maps.append({
            "kvt": kvt_h, "qt": qt_h, "wq": wq_h, "wk": wk_h, "wv": wv_h,
            "wo": wo_h, "bq": bq_h, "bv": bv_h,
            "xq": np.ascontiguousarray(query[b, rows] + bo),
            "gamma": gam_h, "beta": bet_h,
        })

    res = run_bass_kernel_spmd(_get(), in_maps, list(range(N_CORES)))
    out = np.concatenate([r["out"] for r in res.results], axis=0)
    return out.reshape(B, LQ, DQ)


# revision 3
# speedup vs baseline: 1.0104x; 1.0104x over previous
"""Cross-attention layer (B=2, L=2048, D=1024, 16 heads) on 8 TRN2 NeuronCores.

Single-phase fp8 design. Core c handles batch b=c//4 and q rows
512*(c%4)..512*(c%4+1). Each core redundantly projects K,V for its whole
batch in fp8 DoubleRow matmuls (hidden under the softmax exp, which
saturates the Activation engine), then runs scores / ctx / output
projection also as fp8 DoubleRow.

Numerics (validated vs f64 reference on host, max_rel ~1e-3 << 2e-2):
  - all matmul operands fp8e4m3; psum accumulation f32
  - bk dropped entirely (softmax is shift-invariant along kv)
  - bv folded in post-normalize (sum of attention weights == 1)
  - ctx scaled by 16 before fp8 quantization (avoids e4m3 denormals);
    Wo pre-divided by 16 on host; the 16 comes free because the V
    augmentation column holds 1/16 (denominator = den/16)
DoubleRow ISA constraints found by probing walrus: output tile_position
column must be 0 and M must be a multiple of 16 — hence the ones-
augmented V padded to M=80 (64 ctx dims + den row + 15 zero rows), one
psum bank per head, and a 64-partition ctxt layout (one head per slot)
so nothing ever needs a partition remap.
Scores trick: head-dim contraction (64) is split [32, 2] with d in
[0,32) on partitions 32*(h%4).. and d in [32,64) at DR free offset j=1;
the K/Q projections produce this layout directly via host-permuted
weight columns (row tile_position offsets are legal in DR mode).
"""

import numpy as np
import ml_dtypes

import concourse.mybir as mybir
import concourse.tile as tile
from concourse import bacc
from concourse.bass_utils import run_bass_kernel_spmd

dt = mybir.dt
AF = mybir.ActivationFunctionType
ALU = mybir.AluOpType
DR = mybir.MatmulPerfMode.DoubleRow
F8 = ml_dtypes.float8_e4m3

P = 128
B, LQ, LKV = 2, 2048, 2048
DQ, DKV, HID, NH = 1024, 1024, 1024, 16
HD = HID // NH
EPS = 1e-5
N_CORES = 8
RQ = LQ * B // N_CORES             # 512 q rows per core
NPAIR = NH // 2                    # 8 head pairs
KV_C = LKV // P                    # 16 kv chunks of 128
KV_N = LKV // 512                  # 4 kv slices of 512
MQ = RQ // P                       # 4 q tiles of 128
VA = 80                            # augmented V columns (64 + den + pad)


def build():
    nc = bacc.Bacc("TRN2", target_bir_lowering=False, debug=False,
                   num_devices=N_CORES)
    f32, f8 = dt.float32, dt.float8e4
    f32r = dt.float32r

    kvt_d = nc.dram_tensor("kvt", [P, KV_N * 8 * 512], f8, kind="ExternalInput")
    qt_d = nc.dram_tensor("qt", [P, 8 * RQ], f8, kind="ExternalInput")
    wq_d = nc.dram_tensor("wq", [P, 8 * 4 * 2 * P], f8, kind="ExternalInput")
    wk_d = nc.dram_tensor("wk", [P, 8 * 4 * 2 * P], f8, kind="ExternalInput")
    wv_d = nc.dram_tensor("wv", [P, 2 * 4 * 2 * 512], f8, kind="ExternalInput")
    wo_d = nc.dram_tensor("wo", [64, 8 * 2 * DQ], f8, kind="ExternalInput")
    bq_d = nc.dram_tensor("bq", [P, 8], f32, kind="ExternalInput")
    bv_d = nc.dram_tensor("bv", [64, NH], f32, kind="ExternalInput")
    xq_d = nc.dram_tensor("xq", [RQ, DQ], f32, kind="ExternalInput")
    gam_d = nc.dram_tensor("gamma", [1, DQ], f32r, kind="ExternalInput")
    bet_d = nc.dram_tensor("beta", [1, DQ], f32r, kind="ExternalInput")
    out_d = nc.dram_tensor("out", [RQ, DQ], dt.bfloat16,
                           kind="ExternalOutput")

    kvt_r = kvt_d.ap().rearrange("p (n c k) -> n p c k", n=KV_N, c=8)
    qt_r = qt_d.ap().rearrange("p (c k) -> p c k", c=8)
    wq_r = wq_d.ap().rearrange("p (x t i m) -> x p t i m", x=8, t=4, i=2)
    wk_r = wk_d.ap().rearrange("p (x t i m) -> x p t i m", x=8, t=4, i=2)
    wv_r = wv_d.ap().rearrange("p (x t i m) -> x p t i m", x=2, t=4, i=2)
    wo_r = wo_d.ap().rearrange("p (t i m) -> p t i m", t=8, i=2)
    xq_r = xq_d.ap().rearrange("(m p) e -> m p e", p=P)
    out_r = out_d.ap().rearrange("(m p) e -> m p e", p=P)

    with tile.TileContext(nc) as tc:
        with (
            tc.tile_pool(name="const", bufs=1) as const,
            tc.tile_pool(name="epool", bufs=3) as epool,
            tc.tile_pool(name="nm", bufs=2) as nm,
            tc.tile_pool(name="sm", bufs=2) as sm,
            tc.tile_pool(name="opool", bufs=2) as opool,
            tc.tile_pool(name="ps_sc", bufs=2, space="PSUM") as ps_sc,
            tc.tile_pool(name="ctx_ps", bufs=2, space="PSUM") as ctx_ps,
            tc.tile_pool(name="proj_ps", bufs=2, space="PSUM") as proj_ps,
        ):
            # ---- resident SBUF tensors ----
            kvt_sb = const.tile([P, KV_N, 8, 512], f8)
            qt_sb = const.tile([P, 8, RQ], f8)
            wq_sb = const.tile([P, 8, 4, 2, P], f8)
            wk_sb = const.tile([P, 8, 4, 2, P], f8)
            wv_sb = const.tile([P, 2, 4, 2, 512], f8)
            wo_sb = const.tile([64, 8, 2, DQ], f8)
            k_sb = const.tile([P, 4, 2, LKV], f8)       # [32a+d', hg, j, kv]
            q_sb = const.tile([P, 4, 2, RQ], f8)        # [32a+d', hg, j, q]
            v_sb = const.tile([P, 8, 2, NH, VA], f8)    # [kv%128, pr, hf, h, .]
            ctxt_sb = const.tile([64, NH, RQ], f8)      # [d, h, q] (scaled 16x)
            xq_sb = const.tile([P, MQ, DQ], f32)
            bq_sb = const.tile([P, 8], f32)
            bv_sb = const.tile([64, NH], f32)           # 16*bv, [d, h]
            gb_bc = const.tile([P, 2, DQ], f32)
            eps_t = const.tile([P, 1], f32)
            nc.vector.memset(eps_t[:], EPS)

            # ---- DMAs in priority order (serialize on the DMA engines) ----
            nc.sync.dma_start(kvt_sb[:, 0, 0:2], kvt_r[0][:, 0:2])
            nc.sync.dma_start(wk_sb[:, 0], wk_r[0])
            nc.sync.dma_start(qt_sb[:, 0:2], qt_r[:, 0:2])
            nc.sync.dma_start(wq_sb[:, 0], wq_r[0])
            nc.sync.dma_start(bq_sb[:], bq_d.ap())
            nc.sync.dma_start(kvt_sb[:, 0, 2:8], kvt_r[0][:, 2:8])
            nc.sync.dma_start(qt_sb[:, 2:8], qt_r[:, 2:8])
            nc.sync.dma_start(wk_sb[:, 1], wk_r[1])
            nc.sync.dma_start(wq_sb[:, 1], wq_r[1])
            nc.sync.dma_start(wv_sb[:, 0], wv_r[0])
            for n in range(1, KV_N):
                nc.sync.dma_start(kvt_sb[:, n], kvt_r[n])
            for x in range(2, 8):
                nc.sync.dma_start(wq_sb[:, x], wq_r[x])
                nc.sync.dma_start(wk_sb[:, x], wk_r[x])
            nc.sync.dma_start(wv_sb[:, 1], wv_r[1])
            nc.sync.dma_start(bv_sb[:], bv_d.ap())
            nc.sync.dma_start(wo_sb[:], wo_r)
            for m in range(MQ):
                nc.sync.dma_start(xq_sb[:, m], xq_r[m])
            for i, rd in enumerate((gam_d, bet_d)):
                row = sm.tile([1, DQ], f32r, tag="gbrow", name=f"gbrow{i}")
                nc.sync.dma_start(row[:], rd.ap())
                nc.gpsimd.partition_broadcast(gb_bc[:, i, :],
                                              row[:].bitcast(f32))

            # ---- projection emitters ----
            def q_proj(hg, j, pool):
                idx = 2 * hg + j
                ps = pool.tile([P, 512], f32, tag="sc", name=f"q{hg}{j}")
                for t in range(4):
                    nc.tensor.matmul(ps, wq_sb[:, idx, t],
                                     qt_sb[:, 2 * t:2 * t + 2],
                                     start=(t == 0), stop=(t == 3), perf_mode=DR)
                nc.vector.tensor_scalar(q_sb[:, hg, j], ps,
                                        bq_sb[:, idx:idx + 1], None, op0=ALU.add)

            def k_proj(hg, j, n, pool):
                idx = 2 * hg + j
                ps = pool.tile([P, 512], f32, tag="sc", name=f"k{hg}{j}{n}")
                for t in range(4):
                    nc.tensor.matmul(ps, wk_sb[:, idx, t],
                                     kvt_sb[:, n, 2 * t:2 * t + 2],
                                     start=(t == 0), stop=(t == 3), perf_mode=DR)
                nc.vector.tensor_copy(k_sb[:, hg, j, 512 * n:512 * (n + 1)], ps)

            def v_proj(ch, colh, pool):
                n, r = divmod(ch, 4)
                ps = pool.tile([P, 512], f32, tag="sc", name=f"v{ch}{colh}")
                for t in range(4):
                    nc.tensor.matmul(
                        ps, kvt_sb[:, n, 2 * t:2 * t + 2, P * r:P * (r + 1)],
                        wv_sb[:, colh, t], start=(t == 0), stop=(t == 3),
                        perf_mode=DR)
                nc.vector.tensor_copy(
                    v_sb[:, ch // 2, ch % 2, 8 * colh:8 * (colh + 1), 0:64], ps)

            # ---- prologue: minimum to start attention on hp0 ----
            k_proj(0, 0, 0, proj_ps)
            q_proj(0, 0, ps_sc)
            q_proj(0, 1, ps_sc)
            k_proj(0, 1, 0, proj_ps)
            # V augmentation columns: 64 = 1/16 (denominator), 65.. = 0
            # (on Pool so they don't block the prologue psum copies on DVE)
            for pr8 in range(8):
                for hf in range(2):
                    nc.gpsimd.memset(v_sb[:, pr8, hf, :, 64:VA], 0.0)
                    nc.gpsimd.memset(v_sb[:, pr8, hf, :, 64:65], 1.0 / 16.0)

            # remaining projection work, need-ordered, drained per score
            # batch with a deadline-verified quota schedule
            pending = []
            pending += [lambda ch=ch: v_proj(ch, 0, proj_ps) for ch in (0, 1)]
            pending += [lambda: k_proj(0, 0, 1, proj_ps),
                        lambda: k_proj(0, 1, 1, proj_ps)]
            pending += [lambda ch=ch: v_proj(ch, 0, proj_ps)
                        for ch in (2, 3, 4, 5)]
            pending += [lambda: k_proj(0, 0, 2, proj_ps),
                        lambda: k_proj(0, 1, 2, proj_ps)]
            pending += [lambda ch=ch: v_proj(ch, 0, proj_ps)
                        for ch in (6, 7, 8, 9)]
            pending += [lambda: k_proj(0, 0, 3, proj_ps),
                        lambda: k_proj(0, 1, 3, proj_ps)]
            pending += [lambda ch=ch: v_proj(ch, 0, proj_ps)
                        for ch in (10, 11, 12, 13, 14, 15)]
            for hg in range(1, 4):
                pending += [lambda hg=hg: q_proj(hg, 0, proj_ps),
                            lambda hg=hg: q_proj(hg, 1, proj_ps)]
                for n in range(KV_N):
                    pending += [lambda hg=hg, n=n: k_proj(hg, 0, n, proj_ps),
                                lambda hg=hg, n=n: k_proj(hg, 1, n, proj_ps)]
                if hg >= 2:
                    lo = 6 if hg == 3 else 0
                    pending += [lambda ch=ch: v_proj(ch, 1, proj_ps)
                                for ch in range(lo, lo + 6 if hg == 2 else KV_C)]
            pi = [0]
            quota = ([2, 2, 2, 2, 2, 2, 1, 1, 2, 1, 1, 1, 1, 1, 1, 1]
                     + [1] * 112)

            def drain_pending(k):
                for _ in range(k):
                    if pi[0] < len(pending):
                        pending[pi[0]]()
                        pi[0] += 1

            # ---- attention: sequential head pairs; ctx matmuls delayed one
            # score batch so they never separate an unfinished exp from the
            # next scores in the in-order PE stream ----
            batch = 0
            for hp in range(NPAIR):
                hg = hp // 2
                ctxs = [ctx_ps.tile([VA, RQ], f32, tag="ctx", name=f"ctx{s}")
                        for s in range(2)]
                ctx_q = []

                def emit_ctx(pr, e_t, hp=hp, ctxs=ctxs):
                    for hh in range(2):
                        h = 2 * hp + hh
                        nc.tensor.matmul(
                            ctxs[hh][:], v_sb[:, pr, :, h],
                            e_t[:, hh], start=(pr == 0), stop=(pr == 7),
                            perf_mode=DR)

                for pr in range(8):
                    e_t = epool.tile([P, 2, 2, RQ], f8, tag="e")
                    for cpar in range(2):
                        c = 2 * pr + cpar
                        ps_s = ps_sc.tile([P, 2, RQ], f32, tag="sc")
                        for hh in range(2):
                            a = (2 * hp + hh) % 4
                            nc.tensor.matmul(
                                ps_s[:, hh],
                                k_sb[32 * a:32 * (a + 1), hg, :,
                                     P * c:P * (c + 1)],
                                q_sb[32 * a:32 * (a + 1), hg],
                                start=True, stop=True, perf_mode=DR,
                                tile_position=(32 * a, 0))
                        nc.scalar.activation(e_t[:, :, cpar], ps_s[:],
                                             AF.Exp, scale=1.0 / np.sqrt(HD))
                        if ctx_q:
                            ctx_q.pop(0)()
                        drain_pending(quota[batch])
                        batch += 1
                        if cpar == 1:
                            ctx_q.append(
                                lambda pr=pr, e_t=e_t: emit_ctx(pr, e_t))
                for fn in ctx_q:
                    fn()

                # normalize: ctxt = ctx * (16/den) + 16*bv, quantized fp8.
                # All at partitions 0..63; stage-wise so the ctx psum banks
                # free quickly (copies+recips first) for the next head pair.
                halves = ((0, RQ),) if hp < NPAIR - 1 else \
                    ((0, RQ // 2), (RQ // 2, RQ))
                for qlo, qhi in halves:
                    cps, recs, rbs = [], [], []
                    for hh in range(2):
                        rec = sm.tile([1, RQ], f32, tag="rec",
                                      name=f"rec{hh}{qlo}", bufs=4)
                        nc.vector.reciprocal(rec[:, 0:qhi - qlo],
                                             ctxs[hh][64:65, qlo:qhi])
                        recs.append(rec)
                    for hh in range(2):
                        rb = nm.tile([64, RQ], f32, tag="rb",
                                     name=f"rb{hh}{qlo}")
                        nc.gpsimd.partition_broadcast(
                            rb[:, 0:qhi - qlo], recs[hh][:, 0:qhi - qlo])
                        rbs.append(rb)
                        cp = nm.tile([64, RQ], f32, tag="cp",
                                     name=f"cp{hh}{qlo}")
                        nc.vector.tensor_copy(cp[:, 0:qhi - qlo],
                                              ctxs[hh][0:64, qlo:qhi])
                        cps.append(cp)
                    for hh in range(2):
                        h = 2 * hp + hh
                        tmp = nm.tile([64, RQ], f32, tag="tmp",
                                      name=f"tmp{hh}{qlo}")
                        nc.vector.tensor_tensor(tmp[:, 0:qhi - qlo],
                                                cps[hh][:, 0:qhi - qlo],
                                                rbs[hh][:, 0:qhi - qlo],
                                                op=ALU.mult)
                        nc.vector.tensor_scalar(ctxt_sb[:, h, qlo:qhi],
                                                tmp[:, 0:qhi - qlo],
                                                bv_sb[:, h:h + 1], None,
                                                op0=ALU.add)

            # ---- output projection + residual + LayerNorm ----
            # four concurrent psum groups (borrowing the idle proj/ctx pools)
            # so all four q tiles project and LN-pipeline together
            xs, mus, m2s, vars_ = [], [], [], []
            opg = {}
            opg[0] = [ps_sc.tile([P, 2, 512], f32, tag="sc", name="ops0")]
            opg[1] = [ps_sc.tile([P, 2, 512], f32, tag="sc", name="ops1")]
            opg[2] = [proj_ps.tile([P, 512], f32, tag="sc", name=f"ops2{n}")
                      for n in range(2)]
            opg[3] = [ctx_ps.tile([P, RQ], f32, tag="ctx", name=f"ops3{n}")
                      for n in range(2)]
            def ops_ap(m, n2):
                g = opg[m]
                return g[0][:, n2] if len(g) == 1 else g[n2][:]
            for t in range(8):
                for m in range(MQ):
                    for n2 in range(2):
                        nc.tensor.matmul(
                            ops_ap(m, n2),
                            ctxt_sb[:, 2 * t:2 * t + 2, P * m:P * (m + 1)],
                            wo_sb[:, t, :, 512 * n2:512 * (n2 + 1)],
                            start=(t == 0), stop=(t == 7), perf_mode=DR)
            for m in range(MQ):
                x = opool.tile([P, DQ], f32, tag="x", name=f"x{m}", bufs=4)
                mu = sm.tile([P, 1], f32, tag="mu", name=f"mu{m}", bufs=4)
                if len(opg[m]) == 1:
                    src_ap = opg[m][0][:].rearrange("p a b -> p (a b)")
                    nc.vector.scalar_tensor_tensor(
                        x[:], src_ap, 1.0, xq_sb[:, m], op0=ALU.mult,
                        op1=ALU.add, accum_out=mu[:])
                else:
                    mu2 = sm.tile([P, 1], f32, tag="mu2", name=f"mu2{m}",
                                  bufs=2)
                    for n2 in range(2):
                        nc.vector.scalar_tensor_tensor(
                            x[:, 512 * n2:512 * (n2 + 1)], ops_ap(m, n2), 1.0,
                            xq_sb[:, m, 512 * n2:512 * (n2 + 1)],
                            op0=ALU.mult, op1=ALU.add,
                            accum_out=(mu if n2 == 0 else mu2)[:])
                    nc.vector.tensor_tensor(mu[:], mu[:], mu2[:], op=ALU.add)
                xs.append(x)
                mus.append(mu)
            for m in range(MQ):
                xx = opool.tile([P, DQ], f32, tag="xx", name=f"xx{m}", bufs=1)
                m2 = sm.tile([P, 1], f32, tag="m2", name=f"m2{m}", bufs=4)
                nc.scalar.activation(xx[:], xs[m][:], AF.Square,
                                     accum_out=m2[:])
                nc.vector.tensor_scalar(mus[m][:], mus[m][:], 1.0 / DQ, None,
                                        op0=ALU.mult)
                m2s.append(m2)
            for m in range(MQ):
                musq = sm.tile([P, 1], f32, tag="musq", name=f"musq{m}", bufs=4)
                nc.vector.tensor_tensor(musq[:], mus[m][:], mus[m][:],
                                        op=ALU.mult)
                var = sm.tile([P, 1], f32, tag="var", name=f"var{m}", bufs=4)
                nc.vector.tensor_scalar(var[:], m2s[m][:], 1.0 / DQ, None,
                                        op0=ALU.mult)
                nc.vector.tensor_tensor(var[:], var[:], musq[:],
                                        op=ALU.subtract)
                sd = sm.tile([P, 1], f32, tag="sd", name=f"sd{m}", bufs=4)
                nc.scalar.activation(sd[:], var[:], AF.Sqrt, bias=eps_t[:])
                vars_.append(sd)
            for m in range(MQ):
                rstd = sm.tile([P, 1], f32, tag="rstd", name=f"rstd{m}", bufs=4)
                nc.vector.reciprocal(rstd[:], vars_[m][:])
                y = opool.tile([P, DQ], f32, tag="y", name=f"y{m}", bufs=2)
                nc.vector.scalar_tensor_tensor(
                    y[:], xs[m][:], mus[m][:], gb_bc[:, 0], op0=ALU.subtract,
                    op1=ALU.mult)
                z2 = opool.tile([P, DQ], dt.bfloat16, tag="z2", name=f"z2{m}",
                                bufs=2)
                nc.vector.scalar_tensor_tensor(
                    z2[:], y[:], rstd[:], gb_bc[:, 1], op0=ALU.mult,
                    op1=ALU.add)
                nc.sync.dma_start(out_r[m], z2[:])

    nc.compile()
    return nc


_CACHE = {}


def _get():
    if "nc" not in _CACHE:
        _CACHE["nc"] = build()
    return _CACHE["nc"]


def _f8(x):
    return np.ascontiguousarray(np.asarray(x, np.float32)).astype(F8)


def _dr_rows(w):
    """[1024, N] -> [128, 4(t), 2(i), N]: row 128*(2t+i)+p -> (p, t, i)."""
    return np.ascontiguousarray(
        w.reshape(4, 2, P, -1).transpose(2, 0, 1, 3))


# column permutation for the K/Q head-split layout:
# col (hg, j, a, d') = 64*(4*hg+a) + 32*j + d'
_PERM = (64 * (4 * np.arange(4)[:, None, None, None]
               + np.arange(4)[None, None, :, None])
         + 32 * np.arange(2)[None, :, None, None]
         + np.arange(32)[None, None, None, :]).reshape(4, 2, P)


def kernel(query, key_value, Wq, bq, Wk, bk, Wv, bv, Wo, bo, ln_gamma, ln_beta):
    query = np.asarray(query, np.float32)
    key_value = np.asarray(key_value, np.float32)
    Wq = np.asarray(Wq, np.float32)
    Wk = np.asarray(Wk, np.float32)
    Wv = np.asarray(Wv, np.float32)
    Wo = np.asarray(Wo, np.float32)
    bq = np.asarray(bq, np.float32)
    bv = np.asarray(bv, np.float32)
    bo = np.asarray(bo, np.float32)

    # weights in DoubleRow layouts (host-side, shared by all cores)
    wq_h = _dr_rows(Wq)[:, :, :, _PERM]      # [128, 4, 2, 4, 2, 128]
    wq_h = _f8(wq_h.transpose(0, 3, 4, 1, 2, 5).reshape(P, -1))
    wk_h = _dr_rows(Wk)[:, :, :, _PERM]
    wk_h = _f8(wk_h.transpose(0, 3, 4, 1, 2, 5).reshape(P, -1))
    wv_h = _f8(_dr_rows(Wv).reshape(P, 4, 2, 2, 512)
               .transpose(0, 3, 1, 2, 4).reshape(P, -1))
    # wo rows: [64(d), 8(t), 2(i), 1024] with hid = 64*(2t+i)+d, 1/16 scaled
    wo_h = _f8((Wo / 16.0).reshape(8, 2, 64, DQ)
               .transpose(2, 0, 1, 3).reshape(64, -1))
    # bq in psum layout: (p, idx=(hg,j)) -> bq[256*hg + 64*(p//32) + 32*j + p%32]
    p = np.arange(P)
    hgj = np.arange(8)
    bq_h = np.ascontiguousarray(
        bq[256 * (hgj[None, :] // 2) + 64 * (p[:, None] // 32)
           + 32 * (hgj[None, :] % 2) + (p[:, None] % 32)])
    bv_h = np.ascontiguousarray(16.0 * bv.reshape(NH, 64).T)
    gam_h = np.ascontiguousarray(ln_gamma, np.float32).reshape(1, DQ)
    bet_h = np.ascontiguousarray(ln_beta, np.float32).reshape(1, DQ)

    in_maps = []
    for c in range(N_CORES):
        b, rq = divmod(c, N_CORES // B)
        rows = slice(RQ * rq, RQ * (rq + 1))
        kvt = _f8(key_value[b].T)            # [1024, 2048]
        # [p, n, c8, k'] with dkv = 128*c8 + p, kv = 512*n + k'
        kvt_h = np.ascontiguousarray(
            kvt.reshape(8, P, KV_N, 512).transpose(1, 2, 0, 3).reshape(P, -1))
        qt = _f8(query[b, rows].T)           # [1024, 512]
        qt_h = np.ascontiguousarray(
            qt.reshape(8, P, RQ).transpose(1, 0, 2).reshape(P, -1))
        in_maps.append({
            "kvt": kvt_h, "qt": qt_h, "wq": wq_h, "wk": wk_h, "wv": wv_h,
            "wo": wo_h, "bq": bq_h, "bv": bv_h,
            "xq": np.ascontiguousarray(query[b, rows] + bo),
            "gamma": gam_h, "beta": bet_h,
        })

    res = run_bass_kernel_spmd(_get(), in_maps, list(range(N_CORES)))
    out = np.concatenate([np.asarray(r["out"], np.float32)
                          for r in res.results], axis=0)
    return out.reshape(B, LQ, DQ)
